# revision 1
# baseline (speedup 1.0000x reference)
"""Trainium2 Bass kernel for an encoder-decoder LSTM (seq2seq).

Model (see problem reference):
  B=1024, S=96, I=128; HE=HD=1024, O=128, HORIZON=24
  encoder: 96-step LSTMCell(I=128 -> H=1024)
  bridge:  h_dec = h_enc @ W_fc.T + b_fc ; c_dec = h_enc @ W_fcc.T + b_fcc
  decoder: 24 autoregressive LSTMCell(O=128 -> H=1024) steps + out = h @ W_out.T + b_out

Sharding: data-parallel over batch, B=1024 -> 128 per core on 8 cores.
Weights replicated; recurrence runs locally per core; no collectives.

ENCODER runs the gate matmuls in fp8 e4m3 with DoubleRow perf mode (2 fp8
K-rows per PE cell -> 2x column throughput):
  - the 1152-deep contraction (x:128 + h:1024) is split into 5 DoubleRow
    K-groups of 256: 4 pairs of transposed-h chunks, plus (x, pad) where the
    pad rows carry a ones-row that matmul-accumulates the gate BIAS into
    PSUM for free.
  - weights are pre-scaled by 4 and h by 8 (lifts fp8 subnormals); the
    1/32 unscale rides the activation instruction's scale immediate.
  - gate activations read PSUM directly on the scalar engine (no separate
    bias-add evacuation); cell elementwise state (gates, c) is fp16 for
    2x DVE throughput; accumulation and the y output stay fp32.
  - h is produced in fp16; slices 0-3 are PE-transposed (low latency for
    the next step's first DoubleRow groups) and cast to fp8 on the
    PSUM->SBUF copy; slices 4-7 go through DMA transpose + a DVE cast.
DECODER (24 steps) stays in fp16 — the autoregressive y-feedback loop
amplifies quantization noise (fp8 decoder fails the 2e-2 budget in
simulation; fp8 encoder + fp16 decoder sits at ~2.6e-3).

Numerics (numpy sim vs fp32 reference): rel_err ~2.6e-3 (budget 2e-2).
"""

import numpy as np

# ---- model dims (hardcoded; kernel.py must be self-contained) ----
B, S, I = 1024, 96, 128
H = 1024          # HE == HD
O = 128
HORIZON = 24
NCORES = 8
BC = B // NCORES  # 128 batch rows per core
P = 128           # partitions
KH = H // P       # 8 hidden K-chunks
NK = KH + 1       # +1 for the input chunk (decoder fp16 layout)
NG = KH // 2 + 1  # 5 DoubleRow K-groups (encoder fp8 layout)
G = 4 * H         # 4096 gate columns, torch order [i | f | g | o]

SW = 4.0          # encoder W_hh fp8 pre-scale
SHS = 8.0         # encoder h fp8 pre-scale
SA = SW * SHS     # total gate pre-activation scale (32)
ISA = 1.0 / SA


def _build_bass(s_steps=S, horizon=HORIZON):
    import concourse.bass as bass
    import concourse.tile as tile
    from concourse import bacc, mybir

    f8 = mybir.dt.float8e4
    f16 = mybir.dt.float16
    f32 = mybir.dt.float32
    ADD = mybir.AluOpType.add
    MULT = mybir.AluOpType.mult
    SIG = mybir.ActivationFunctionType.Sigmoid
    TANH = mybir.ActivationFunctionType.Tanh
    IDENT = mybir.ActivationFunctionType.Identity
    DRMODE = mybir.MatmulPerfMode.DoubleRow

    import os

    nc = bacc.Bacc()
    _trace_sim = os.environ.get("BASS_TRACE_SIM", "0") == "1"

    # xg: per-step stationary operand for the x+bias DoubleRow group:
    #   [:, 0, :] = x_t transposed [I, BC] (fp8)
    #   [:, 1, :] = ones in partition 0, zeros elsewhere (bias rider)
    xg_d = nc.dram_tensor("xg", [s_steps, P, 2, BC], f8, kind="ExternalInput")
    # ew8: encoder weights+bias in DoubleRow K-group layout [P, NG, 2, G]
    ew8_d = nc.dram_tensor("ew8", [P, NG, 2, G], f8, kind="ExternalInput")
    dw_d = nc.dram_tensor("dw", [P, NK, G], f16, kind="ExternalInput")
    bw_d = nc.dram_tensor("bw", [P, KH, 2 * H], f16, kind="ExternalInput")
    ow_d = nc.dram_tensor("ow", [P, KH, O], f16, kind="ExternalInput")
    bd_d = nc.dram_tensor("bd", [G], f16, kind="ExternalInput")
    bbr_d = nc.dram_tensor("bbr", [2 * H], f16, kind="ExternalInput")
    bo_d = nc.dram_tensor("bo", [O], f32, kind="ExternalInput")
    y_d = nc.dram_tensor("y", [horizon, O, BC], f32, kind="ExternalOutput")

    def bcast_rows(ap):
        # DRAM [N] -> read the same row on all 128 partitions
        return bass.AP(tensor=ap.tensor, offset=ap.offset, ap=[[0, P], *ap.ap])

    with tile.TileContext(nc, trace_sim=_trace_sim) as tc:
        with (
            tc.tile_pool(name="consts", bufs=1) as consts,
            tc.tile_pool(name="wpool", bufs=1) as wpool,
            tc.tile_pool(name="w2pool", bufs=1) as w2pool,
            tc.tile_pool(name="xgpool", bufs=3) as xgpool,
            tc.tile_pool(name="inpool", bufs=2) as inpool,
            tc.tile_pool(name="hpool", bufs=2) as hpool,
            tc.tile_pool(name="state", bufs=1) as state,
            tc.tile_pool(name="ypool", bufs=2) as ypool,
            tc.tile_pool(name="gpsum", bufs=3, space="PSUM") as gpsum,
            tc.tile_pool(name="trpool", bufs=2, space="PSUM") as trpool,
        ):
            # ---- encoder weights first: group 0 feeds the prologue ----
            ew8_sb = wpool.tile([P, NG, 2, G], f8, tag="w8", name="ew8")
            for g in range(NG):
                nc.sync.dma_start(out=ew8_sb[:, g], in_=ew8_d[:, g])

            # ---- constants ----
            bd_sb = consts.tile([P, G], f16)
            nc.gpsimd.dma_start(out=bd_sb, in_=bcast_rows(bd_d[:]))
            bbr_sb = consts.tile([P, 2 * H], f16)
            nc.gpsimd.dma_start(out=bbr_sb, in_=bcast_rows(bbr_d[:]))
            bo_sb = consts.tile([P, 1], f32)
            nc.sync.dma_start(out=bo_sb, in_=bo_d[:][:, None])
            bw_sb = consts.tile([P, KH, 2 * H], f16)
            nc.sync.dma_start(out=bw_sb, in_=bw_d[:])
            ow_sb = consts.tile([P, KH, O], f16)
            nc.sync.dma_start(out=ow_sb, in_=ow_d[:])
            ident = consts.tile([P, P], f16)
            from concourse.masks import make_identity
            make_identity(nc, ident)
            # decoder o-gate bias rider (matmul-accumulated for slices 0-2)
            ones_sb = consts.tile([P, P], f16)
            nc.vector.memset(ones_sb, 1.0)
            obias_d = consts.tile([P, 3 * P], f16)
            nc.vector.memset(obias_d, 0.0)
            nc.vector.tensor_copy(out=obias_d[0:1, :], in_=bd_sb[0:1, 3 * H : 3 * H + 3 * P])

            # ---- persistent state (fp16 gates/cell for 2x DVE) ----
            c_sb = state.tile([P, H], f16)       # cell state, [B, H]
            pre = state.tile([P, G], f16)        # gate post-activations
            cf = state.tile([P, H], f16)
            ig = state.tile([P, H], f16)
            thc = state.tile([P, H], f16)        # tanh(c)

            def alloc_pair():
                return gpsum.tile([P, H], f32, tag="g", name="gps")

            # ================= encoder (fp8 DoubleRow) =================

            def emit_dr(pst, pair, lhsT3, g, start, stop):
                for hh in range(2):
                    col = pair * H + hh * 512
                    nc.tensor.matmul(
                        pst[:, hh * 512 : hh * 512 + 512],
                        lhsT=lhsT3, rhs=ew8_sb[:, g, :, col : col + 512],
                        start=start, stop=stop, perf_mode=DRMODE,
                    )

            def load_xg(t):
                xt = xgpool.tile([P, 2, BC], f8, tag="xg", name="xg")
                nc.sync.dma_start(out=xt, in_=xg_d[t])
                return xt

            def emit_if_groups(ps, hT8):
                # pairs i, f over the 4 h DoubleRow groups, group-interleaved
                # so hT8 chunk pairs are consumed in production order
                for j in range(4):
                    lhs = hT8[:, 2 * j : 2 * j + 2, :]
                    for pair in (0, 1):
                        emit_dr(ps[pair], pair, lhs, 1 + j,
                                start=False, stop=(j == 3))

            def emit_dr_hh(pst, pair, hh, lhsT3, g, start, stop):
                col = pair * H + hh * 512
                nc.tensor.matmul(
                    pst[:, hh * 512 : hh * 512 + 512],
                    lhsT=lhsT3, rhs=ew8_sb[:, g, :, col : col + 512],
                    start=start, stop=stop, perf_mode=DRMODE,
                )

            def emit_cell8(pi, pf, pg, po, first, last):
                """fp8-encoder cell for the hh-split schedule: the g/o gate
                halves stop early (phase-2 is ordered g-hh0, o-hh0, g-hh1,
                o-hh1), so activations, the c chain and h-production all run
                WHILE the remaining matmuls stream; the 8 PE transposes queue
                after the last matmul and find their h slices ready.

                PSUM reads that gate a buffer reuse (i/f rests) are evacuated
                on DVE (tensor_scalar with the 1/SA unscale) so the frees
                never wait behind the scalar queue; everything else activates
                straight from PSUM on the scalar engine."""
                s0 = slice(0, P)
                rsl = slice(P, H)
                sm = slice(P, 4 * P)
                sh = slice(4 * P, H)

                def act_ps(dst_lo, psrc, width, func):
                    nc.scalar.activation(
                        out=pre[:, dst_lo : dst_lo + width], in_=psrc,
                        func=func, scale=ISA,
                    )

                def cell_upd(sl):
                    nc.vector.tensor_tensor(
                        out=ig[:, sl], in0=pre[:, sl],
                        in1=pre[:, 2 * H + sl.start : 2 * H + sl.stop], op=MULT,
                    )
                    if first:
                        nc.vector.tensor_copy(out=c_sb[:, sl], in_=ig[:, sl])
                    else:
                        nc.vector.tensor_tensor(
                            out=c_sb[:, sl], in0=cf[:, sl], in1=ig[:, sl], op=ADD
                        )
                    nc.scalar.activation(out=thc[:, sl], in_=c_sb[:, sl], func=TANH)

                # ---- i, f: s0 direct on ACT, rests evacuated via DVE ----
                act_ps(0, pi[:, s0], P, SIG)
                nc.vector.tensor_scalar_mul(pre[:, P : H], pi[:, rsl], ISA)
                nc.scalar.activation(
                    out=pre[:, P : H], in_=pre[:, P : H], func=SIG
                )
                if not first:
                    act_ps(H, pf[:, s0], P, SIG)
                    nc.gpsimd.tensor_tensor(
                        out=cf[:, s0], in0=pre[:, H : H + P], in1=c_sb[:, s0],
                        op=MULT,
                    )
                    nc.vector.tensor_scalar_mul(pre[:, H + P : 2 * H], pf[:, rsl], ISA)
                    nc.scalar.activation(
                        out=pre[:, H + P : 2 * H], in_=pre[:, H + P : 2 * H],
                        func=SIG,
                    )
                    nc.gpsimd.tensor_tensor(
                        out=cf[:, sm], in0=pre[:, H + sm.start : H + sm.stop],
                        in1=c_sb[:, sm], op=MULT,
                    )
                    nc.gpsimd.tensor_tensor(
                        out=cf[:, sh], in0=pre[:, H + sh.start : H + sh.stop],
                        in1=c_sb[:, sh], op=MULT,
                    )

                # ---- g hh0 -> c chain for s0 + sm (concurrent with o/g hh1 MMs) ----
                act_ps(2 * H, pg[:, s0], P, TANH)
                cell_upd(s0)
                act_ps(2 * H + P, pg[:, sm], 3 * P, TANH)
                cell_upd(sm)

                # ---- h production + transposes ----
                ob = 3 * H
                h_sb = hpool.tile([P, H], f16, tag="h", name="hsb")
                if last:
                    hTn = hpool.tile([P, KH, BC], f16, tag="hT", name="hT")
                else:
                    hTn = hpool.tile([P, KH, BC], f8, tag="hT8", name="hT8")

                def h_slice(s):
                    lo = s * P
                    nc.vector.tensor_tensor(
                        out=h_sb[:, lo : lo + P], in0=pre[:, ob + lo : ob + lo + P],
                        in1=thc[:, lo : lo + P], op=MULT,
                    )

                def pe_tr(s, eng):
                    pst = trpool.tile([P, P], f16, tag="t", name="trp")
                    nc.tensor.transpose(
                        pst[:], h_sb[:, s * P : (s + 1) * P], ident
                    )
                    if last:
                        if eng == "act":
                            nc.scalar.copy(out=hTn[:, s, :], in_=pst[:])
                        else:
                            nc.vector.tensor_copy(out=hTn[:, s, :], in_=pst[:])
                    else:
                        if eng == "act":
                            nc.scalar.activation(
                                out=hTn[:, s, :], in_=pst[:], func=IDENT,
                                scale=SHS,
                            )
                        else:
                            nc.vector.tensor_scalar_mul(hTn[:, s, :], pst[:], SHS)

                # o hh0: slices 0-3
                act_ps(ob, po[:, s0], P, SIG)
                h_slice(0); pe_tr(0, "act")
                act_ps(ob + P, po[:, sm], 3 * P, SIG)
                h_slice(1); pe_tr(1, "vec")
                h_slice(2); pe_tr(2, "act")
                h_slice(3); pe_tr(3, "vec")
                # g hh1 -> c chain sh; o hh1 -> slices 4-7
                act_ps(2 * H + 4 * P, pg[:, sh], H - 4 * P, TANH)
                cell_upd(sh)
                act_ps(ob + 4 * P, po[:, sh], H - 4 * P, SIG)
                h_slice(4); pe_tr(4, "act")
                h_slice(5); pe_tr(5, "vec")
                h_slice(6); pe_tr(6, "act")
                h_slice(7); pe_tr(7, "vec")
                return hTn

            # prologue: pre-emit x+bias matmuls of pairs i, f for t=0
            xt = load_xg(0)
            pi = alloc_pair()
            pf = alloc_pair()
            pg = alloc_pair()
            for pst in (pi, pf):
                pair = 0 if pst is pi else 1
                for hh in (0, 1):
                    emit_dr_hh(pst, pair, hh, xt[:, :, :], 0,
                               start=True, stop=True)
            hT8 = None
            hT_f16 = None
            for t in range(s_steps):
                first = t == 0
                last = t == s_steps - 1
                if not first:
                    # phase 1: pair i fully, then pair f (i stops early so its
                    # PSUM reads can clear before phase-2's buffer reuse)
                    for pair, pst in ((0, pi), (1, pf)):
                        for j in range(4):
                            emit_dr_hh(pst, pair, 0, hT8[:, 2 * j : 2 * j + 2, :],
                                       1 + j, start=False, stop=(j == 3))
                            emit_dr_hh(pst, pair, 1, hT8[:, 2 * j : 2 * j + 2, :],
                                       1 + j, start=False, stop=(j == 3))
                # phase 2, hh-split: g-hh0, o-hh0, g-hh1, o-hh1
                po = alloc_pair()
                for hh in (0, 1):
                    for pair, pst in ((2, pg), (3, po)):
                        emit_dr_hh(pst, pair, hh, xt[:, :, :], 0,
                                   start=True, stop=first)
                        if not first:
                            for j in range(4):
                                emit_dr_hh(pst, pair, hh,
                                           hT8[:, 2 * j : 2 * j + 2, :], 1 + j,
                                           start=False, stop=(j == 3))
                # next-step prefetch: x matmuls for i', f'
                if t + 1 < s_steps:
                    xt = load_xg(t + 1)
                    ni = alloc_pair()
                    nf = alloc_pair()
                    ng = alloc_pair()
                    for pair, pst in ((0, ni), (1, nf)):
                        for hh in (0, 1):
                            emit_dr_hh(pst, pair, hh, xt[:, :, :], 0,
                                       start=True, stop=False)
                else:
                    ni = nf = ng = None
                out_hT = emit_cell8(pi, pf, pg, po, first=first, last=last)
                if last:
                    hT_f16 = out_hT
                else:
                    hT8 = out_hT
                pi, pf, pg = ni, nf, ng

            hT = hT_f16

            # ================= bridge (fp16) =================
            ps_h = alloc_pair()
            ps_c = alloc_pair()
            for k in range(KH):
                for hh in range(2):
                    nc.tensor.matmul(
                        ps_h[:, hh * 512 : hh * 512 + 512],
                        lhsT=hT[:, k, :],
                        rhs=bw_sb[:, k, hh * 512 : hh * 512 + 512],
                        start=(k == 0), stop=(k == KH - 1),
                    )
                    nc.tensor.matmul(
                        ps_c[:, hh * 512 : hh * 512 + 512],
                        lhsT=hT[:, k, :],
                        rhs=bw_sb[:, k, H + hh * 512 : H + hh * 512 + 512],
                        start=(k == 0), stop=(k == KH - 1),
                    )
            # decoder weights load (overlaps the bridge)
            w2_sb = w2pool.tile([P, NK, G], f16, tag="w2", name="w2")
            for k in (*range(1, NK), 0):
                nc.sync.dma_start(out=w2_sb[:, k], in_=dw_d[:, k])

            nc.vector.tensor_tensor(
                out=c_sb[:], in0=ps_c[:], in1=bbr_sb[:, H : 2 * H], op=ADD
            )
            h_sb = hpool.tile([P, H], f16, tag="h", name="hsb")
            nc.vector.tensor_tensor(
                out=h_sb[:], in0=ps_h[:], in1=bbr_sb[:, 0:H], op=ADD
            )
            hT = hpool.tile([P, KH, BC], f16, tag="hT", name="hT")
            for s in range(KH):
                nc.sync.dma_start(
                    out=hT[:, s, :], in_=h_sb[:, s * P : (s + 1) * P],
                    transpose=True,
                )

            # ================= decoder (fp16, as the baseline) =================
            def emit_pair_x(w, pst, pair, xt16, start, stop):
                for hh in range(2):
                    col = pair * H + hh * 512
                    nc.tensor.matmul(
                        pst[:, hh * 512 : hh * 512 + 512],
                        lhsT=xt16, rhs=w[:, 0, col : col + 512],
                        start=start, stop=stop,
                    )

            def emit_if_pairs(w, ps, hTl, stop, po_hook=None):
                # decoder i/f pairs: the h-matmuls open the accumulation
                # (start at k==0); the x feedback matmuls come later with
                # start=False
                for k in range(KH):
                    for pair in (0, 1):
                        for hh in range(2):
                            col = pair * H + hh * 512
                            nc.tensor.matmul(
                                ps[pair][:, hh * 512 : hh * 512 + 512],
                                lhsT=hTl[:, k, :], rhs=w[:, 1 + k, col : col + 512],
                                start=(k == 0),
                                stop=(stop and k == KH - 1),
                            )
                    if po_hook is not None:
                        po_hook(k)

            def emit_obias(pst, obias, stop):
                nc.tensor.matmul(
                    pst[:, 0 : 3 * P], lhsT=ones_sb[:], rhs=obias[:],
                    start=False, stop=stop,
                )

            def emit_pair_h(w, pst, pair, hTl, start, stop):
                for k in range(KH):
                    for hh in range(2):
                        col = pair * H + hh * 512
                        nc.tensor.matmul(
                            pst[:, hh * 512 : hh * 512 + 512],
                            lhsT=hTl[:, k, :], rhs=w[:, 1 + k, col : col + 512],
                            start=(start and k == 0), stop=(stop and k == KH - 1),
                        )

            def emit_cell16(ps, bias_sb):
                """Decoder cell (fp16 path, bias via DVE evac + obias matmul)."""
                s0 = slice(0, P)
                rs = slice(P, H)
                sm = slice(P, 4 * P)
                sh = slice(4 * P, H)

                def evac(dst_lo, psrc, width):
                    nc.vector.tensor_tensor(
                        out=pre[:, dst_lo : dst_lo + width],
                        in0=psrc, in1=bias_sb[:, dst_lo : dst_lo + width], op=ADD,
                    )

                def act(lo, width, func):
                    nc.scalar.activation(
                        out=pre[:, lo : lo + width], in_=pre[:, lo : lo + width],
                        func=func,
                    )

                def cell_upd(sl):
                    nc.vector.tensor_tensor(
                        out=ig[:, sl], in0=pre[:, sl],
                        in1=pre[:, 2 * H + sl.start : 2 * H + sl.stop], op=MULT,
                    )
                    nc.vector.tensor_tensor(
                        out=c_sb[:, sl], in0=cf[:, sl], in1=ig[:, sl], op=ADD
                    )
                    nc.scalar.activation(out=thc[:, sl], in_=c_sb[:, sl], func=TANH)

                # ---- slice-0 chain ----
                evac(0, ps[0][:, s0], P); act(0, P, SIG)
                evac(H, ps[1][:, s0], P); act(H, P, SIG)
                nc.gpsimd.tensor_tensor(
                    out=cf[:, s0], in0=pre[:, H : H + P], in1=c_sb[:, s0],
                    op=MULT,
                )
                evac(2 * H, ps[2][:, s0], P); act(2 * H, P, TANH)
                cell_upd(s0)

                # ---- full-width rest for i, f, g; c chain middle ----
                evac(P, ps[0][:, rs], H - P); act(P, H - P, SIG)
                evac(H + P, ps[1][:, rs], H - P); act(H + P, H - P, SIG)
                nc.gpsimd.tensor_tensor(
                    out=cf[:, rs], in0=pre[:, H + P : 2 * H], in1=c_sb[:, rs],
                    op=MULT,
                )
                evac(2 * H + P, ps[2][:, rs], H - P); act(2 * H + P, H - P, TANH)
                cell_upd(sm)

                # ---- o gate tail ----
                ob = 3 * H
                h_sb = hpool.tile([P, H], f16, tag="h", name="hsb")
                hT_new = hpool.tile([P, KH, BC], f16, tag="hT", name="hT")

                def h_slice(s):
                    sl = slice(s * P, (s + 1) * P)
                    nc.vector.tensor_tensor(
                        out=h_sb[:, sl], in0=pre[:, ob + s * P : ob + (s + 1) * P],
                        in1=thc[:, sl], op=MULT,
                    )

                def pe_tr(s):
                    pst = trpool.tile([P, P], f16, tag="t", name="trp")
                    nc.tensor.transpose(
                        pst[:], h_sb[:, s * P : (s + 1) * P], ident
                    )
                    nc.scalar.copy(out=hT_new[:, s, :], in_=pst[:])

                # slice 0: sigmoid straight from PSUM (o bias was
                # matmul-accumulated for slices 0-2)
                nc.scalar.activation(
                    out=pre[:, ob : ob + P], in_=ps[3][:, s0], func=SIG
                )
                h_slice(0); pe_tr(0)
                nc.scalar.activation(
                    out=pre[:, ob + P : ob + 3 * P], in_=ps[3][:, P : 3 * P],
                    func=SIG,
                )
                h_slice(1); pe_tr(1)
                h_slice(2); pe_tr(2)
                evac(ob + 3 * P, ps[3][:, 3 * P : H], H - 3 * P)
                act(ob + 3 * P, H - 3 * P, SIG)
                cell_upd(sh)
                for s in range(3, KH):
                    h_slice(s)
                    nc.sync.dma_start(
                        out=hT_new[:, s, :],
                        in_=h_sb[:, s * P : (s + 1) * P], transpose=True,
                    )
                return hT_new

            # reference order: cell first (inp from the previous step,
            # zeros at t=0), then project the NEW h:
            #   y[t] = h_{t+1} @ W_out.T + b_out
            inpT = None
            pend_po = None       # (po_tile, hT_tile) awaiting projection
            for t in range(horizon):
                first = t == 0
                ps = {p: alloc_pair() for p in (0, 1, 2, 3)}

                po_hook = None
                if pend_po is not None:
                    ppo, phT = pend_po

                    def po_hook(k, ppo=ppo, phT=phT):
                        nc.tensor.matmul(
                            ppo[:, 0:BC], lhsT=ow_sb[:, k, :],
                            rhs=phT[:, k, :],
                            start=(k == 0), stop=(k == KH - 1),
                        )

                emit_if_pairs(w2_sb, ps, hT, stop=first, po_hook=po_hook)
                if pend_po is not None:
                    ppo, _ = pend_po
                    y_sb = ypool.tile([P, BC], f32, tag="y", name="ysb")
                    nc.scalar.activation(
                        out=y_sb[:], in_=ppo[:, 0:BC], func=IDENT, bias=bo_sb[:]
                    )
                    nc.sync.dma_start(out=y_d[t - 1], in_=y_sb[:])
                    inpT = inpool.tile([P, BC], f16, tag="inpT", name="inpT")
                    nc.gpsimd.tensor_copy(out=inpT[:], in_=y_sb[:])
                emit_pair_h(w2_sb, ps[2], 2, hT, start=True, stop=first)
                if not first:
                    for p in (0, 1, 2):
                        emit_pair_x(w2_sb, ps[p], p, inpT, start=False,
                                    stop=True)
                emit_pair_h(w2_sb, ps[3], 3, hT, start=True, stop=False)
                emit_obias(ps[3], obias_d, stop=first)
                if not first:
                    emit_pair_x(w2_sb, ps[3], 3, inpT, start=False, stop=True)
                hT = emit_cell16(ps, bd_sb)
                pend_po = (trpool.tile([P, BC], f32, tag="t", name="po"), hT)

            # final step's projection
            ppo, phT = pend_po
            for k in range(KH):
                nc.tensor.matmul(
                    ppo[:, 0:BC], lhsT=ow_sb[:, k, :], rhs=phT[:, k, :],
                    start=(k == 0), stop=(k == KH - 1),
                )
            y_sb = ypool.tile([P, BC], f32, tag="y", name="ysb")
            nc.scalar.activation(
                out=y_sb[:], in_=ppo[:, 0:BC], func=IDENT, bias=bo_sb[:]
            )
            nc.sync.dma_start(out=y_d[horizon - 1], in_=y_sb[:])

    nc.compile()
    return nc


def _prepare_inputs(inputs, s_steps=S):
    import ml_dtypes

    f16 = np.float16
    E4 = ml_dtypes.float8_e4m3   # TRN fp8e4 variant (max +-240)
    x = np.asarray(inputs["x"], np.float32)[:, :s_steps]
    W_ih_e = np.asarray(inputs["W_ih_e"], np.float32)
    W_hh_e = np.asarray(inputs["W_hh_e"], np.float32)
    W_ih_d = np.asarray(inputs["W_ih_d"], np.float32)
    W_hh_d = np.asarray(inputs["W_hh_d"], np.float32)
    W_fc = np.asarray(inputs["W_fc"], np.float32)
    W_fcc = np.asarray(inputs["W_fcc"], np.float32)
    W_out = np.asarray(inputs["W_out"], np.float32)

    def kmajor(wT, nk, ncol):
        # [nk*128, ncol] -> [128, nk, ncol]
        return np.ascontiguousarray(
            wT.reshape(nk, P, ncol).transpose(1, 0, 2).astype(f16)
        )

    # ---- encoder fp8 DoubleRow weight layout [P, NG, 2, G] ----
    be32 = (np.asarray(inputs["b_ih_e"], np.float32)
            + np.asarray(inputs["b_hh_e"], np.float32))
    WhhT = W_hh_e.T * SW                     # [H, G], pre-scaled
    WihT = W_ih_e.T * SA                     # [I, G]
    ew8 = np.zeros((P, NG, 2, G), np.float32)
    ew8[:, 0, 0, :] = WihT
    ew8[0, 0, 1, :] = be32 * SA              # bias rider row
    for j in range(4):
        ew8[:, 1 + j, 0, :] = WhhT[(2 * j) * P : (2 * j + 1) * P]
        ew8[:, 1 + j, 1, :] = WhhT[(2 * j + 1) * P : (2 * j + 2) * P]
    ew8 = ew8.astype(E4)

    dw = kmajor(np.concatenate([W_ih_d.T, W_hh_d.T], axis=0), NK, G)
    bw = kmajor(np.concatenate([W_fc.T, W_fcc.T], axis=1), KH, 2 * H)
    ow = kmajor(W_out.T, KH, O)
    bd = (inputs["b_ih_d"] + inputs["b_hh_d"]).astype(f16)
    bbr = np.concatenate([inputs["b_fc"], inputs["b_fcc"]]).astype(f16)
    bo = np.asarray(inputs["b_out"], np.float32)

    shared = dict(ew8=ew8, dw=dw, bw=bw, ow=ow, bd=bd, bbr=bbr, bo=bo)
    in_maps = []
    for c in range(NCORES):
        xc = x[c * BC : (c + 1) * BC]                    # [BC, S, I]
        xT = xc.transpose(1, 2, 0)                       # [S, I, BC]
        xg = np.zeros((s_steps, P, 2, BC), np.float32)
        xg[:, :, 0, :] = xT
        xg[:, 0, 1, :] = 1.0                             # bias ones row
        in_maps.append(dict(shared, xg=xg.astype(E4)))
    return in_maps


def run(inputs, trace=False, s_steps=S, horizon=HORIZON):
    """Build, run on 8 cores, gather. Returns (full_output, BassKernelResults)."""
    import sys

    try:
        import concourse  # noqa: F401
    except ImportError:
        sys.path.insert(0, "/opt/trn_rl_repo")
    from concourse.bass_utils import run_bass_kernel_spmd

    nc = _build_bass(s_steps=s_steps, horizon=horizon)
    in_maps = _prepare_inputs(inputs, s_steps=s_steps)
    res = run_bass_kernel_spmd(nc, in_maps, core_ids=list(range(NCORES)), trace=trace)
    out = np.empty((B, horizon, O), np.float32)
    for c in range(NCORES):
        yc = res.results[c]["y"]                         # [horizon, O, BC]
        out[c * BC : (c + 1) * BC] = yc.transpose(2, 0, 1)
    return out, res


def kernel(**inputs):
    out, _ = run(inputs, trace=False)
    return out

